# revision 8
# baseline (speedup 1.0000x reference)
"""Trainium2 Bass kernel for nn_DrugResponsePrior (embedding_lookup).

Key algebraic fact (guaranteed by the input spec): cell_map values < 100,
is_missing in {0,1}, drug_map values < 256.  Every output row depends only on
(cs, dm) with cs = cell_map[idx] + 100*is_missing[idx] in [0,200) and
dm = drug_map[tidx] in [0,256) -> only 200*256 = 51200 distinct outputs.

Strategy (8 cores):
  * table construction sharded: core c computes the 25 cell-states
    [25c, 25c+25) -> 6400 pairs -> [6400, 9] table shard; AllGather -> full
    [51200, 9] table in DRAM on every core.
  * sample side data-parallel: each core takes 8192 rows of idx/tidx,
    indirect-gathers cm/miss/dm, computes pid = cs*256+dm, and
    indirect-gathers the final 9-float rows from the table.

All model math runs on device; the host only shards/reshapes/transposes
input tensors and concatenates the output.
"""
import sys

if "/opt/trn_rl_repo" not in sys.path:
    sys.path.insert(0, "/opt/trn_rl_repo")

import numpy as np

import concourse.bass as bass
import concourse.mybir as mybir
import concourse.tile as tile
from concourse.bass_utils import run_bass_kernel_spmd
from concourse.masks import make_identity

f32 = mybir.dt.float32
i32 = mybir.dt.int32

# problem sizes (hardcoded per spec)
B = 65536
R = 262144
NCELL100 = 100      # distinct cell_map values
NDRUG = 256
NFEAT = 1024
CEMB = 1024
DEMB = 128
HID = 200
NDOSES = 9
NCORES = 8

BS = B // NCORES            # samples per core = 8192
P = 128
J = BS // P                 # idx per partition = 64
S_SH = 200 // NCORES        # states per core = 25
NPAIR_SH = S_SH * NDRUG     # 6400
NPAIR = 200 * NDRUG         # 51200
EPS = 1e-12

_NC_CACHE = {}


def _split_sync_waits(nc, limit=1):
    """This walrus build accepts at most one sync-wait per instruction; hoist
    excess waits onto same-engine NoOps inserted just before."""
    ctr = 0
    for bb in nc.main_func.blocks:
        new_list = []
        for inst in bb.instructions:
            si = inst.sync_info
            if si is not None and si.on_wait and len(si.on_wait) > limit:
                waits = list(si.on_wait)
                head, tail = waits[:-limit], waits[-limit:]
                for j in range(0, len(head), limit):
                    nop = mybir.InstNoOp(name=f"waitnop-{ctr}", engine=inst.engine)
                    ctr += 1
                    nop.sync_info = mybir.SyncInfo(
                        on_wait=list(head[j : j + limit]), on_update=[]
                    )
                    new_list.append(nop)
                inst.sync_info = mybir.SyncInfo(
                    on_wait=list(tail),
                    on_update=list(si.on_update) if si.on_update else [],
                )
            new_list.append(inst)
        bb.instructions[:] = new_list
    return nc


def build_nc(split_waits=True):
    nc = bass.Bass(num_devices=NCORES)
    AF = mybir.ActivationFunctionType
    ALU = mybir.AluOpType

    # ---------------- kernel I/O ----------------
    idx_s = nc.dram_tensor("idx_s", [BS], i32, kind="ExternalInput")
    tidx_s = nc.dram_tensor("tidx_s", [BS], i32, kind="ExternalInput")
    cell_map = nc.dram_tensor("cell_map", [R], i32, kind="ExternalInput")
    is_missing = nc.dram_tensor("is_missing", [R], i32, kind="ExternalInput")
    drug_map = nc.dram_tensor("drug_map", [R], i32, kind="ExternalInput")
    cfT_s = nc.dram_tensor("cfT_s", [NFEAT, S_SH], f32, kind="ExternalInput")
    me_s = nc.dram_tensor("me_s", [S_SH, CEMB], f32, kind="ExternalInput")
    flag_s = nc.dram_tensor("flag_s", [S_SH], f32, kind="ExternalInput")
    drug_emb = nc.dram_tensor("drug_emb", [NDRUG, DEMB], f32, kind="ExternalInput")
    drug_embT = nc.dram_tensor("drug_embT", [DEMB, NDRUG], f32, kind="ExternalInput")
    W1 = nc.dram_tensor("W1", [NFEAT, CEMB], f32, kind="ExternalInput")
    b1 = nc.dram_tensor("b1", [CEMB], f32, kind="ExternalInput")
    Wf1c = nc.dram_tensor("Wf1c", [CEMB, HID], f32, kind="ExternalInput")
    Wf1d = nc.dram_tensor("Wf1d", [DEMB, HID], f32, kind="ExternalInput")
    bf1 = nc.dram_tensor("bf1", [HID], f32, kind="ExternalInput")
    Wf2 = nc.dram_tensor("Wf2", [HID, HID], f32, kind="ExternalInput")
    bf2 = nc.dram_tensor("bf2", [HID], f32, kind="ExternalInput")
    Wf3 = nc.dram_tensor("Wf3", [HID, NDOSES], f32, kind="ExternalInput")
    bf3 = nc.dram_tensor("bf3", [NDOSES], f32, kind="ExternalInput")
    mu_s = nc.dram_tensor("mu_s", [BS, NDOSES], f32, kind="ExternalOutput")

    # cumsum matrix for softplus rows: Lsp[k,o] = 1 iff k+1 <= o
    Lmat = nc.inline_tensor(np.triu(np.ones((NDOSES - 1, NDOSES), np.float32), 1), name="Lmat")

    # collective staging (plain Internal DRAM tensors; offset-0 APs for
    # the indirect gather)
    cc_in = nc.dram_tensor("cc_in", [NPAIR_SH, NDOSES], f32)
    cc_out = nc.dram_tensor("cc_out", [NPAIR, NDOSES], f32, addr_space="Shared")

    with tile.TileContext(nc) as tc, \
            tc.tile_pool(name="sb", bufs=1) as sb, \
            tc.tile_pool(name="sbw", bufs=1) as sbw:
        with (
            tc.tile_pool(name="ps_tr", bufs=3, space="PSUM") as ps_tr,
            tc.tile_pool(name="ps_ps", bufs=2, space="PSUM") as ps_ps,
            tc.tile_pool(name="ps_aa", bufs=1, space="PSUM") as ps_aa,
            tc.tile_pool(name="ps_bd", bufs=2, space="PSUM") as ps_bd,
        ):
            # ======== sample side: gather cm/miss/dm, compute pid ========
            idx_t = sb.tile([P, J], i32)
            tidx_t = sb.tile([P, J], i32)
            nc.sync.dma_start(out=idx_t[:], in_=idx_s[:].rearrange("(p j) -> p j", p=P))
            nc.sync.dma_start(out=tidx_t[:], in_=tidx_s[:].rearrange("(p j) -> p j", p=P))
            cm_t = sb.tile([P, J], i32)
            mi_t = sb.tile([P, J], i32)
            dm_t = sb.tile([P, J], i32)
            nc.gpsimd.indirect_dma_start(
                out=cm_t[:], out_offset=None,
                in_=cell_map[:].rearrange("(r one) -> r one", one=1),
                in_offset=bass.IndirectOffsetOnAxis(ap=idx_t[:], axis=0))
            nc.gpsimd.indirect_dma_start(
                out=mi_t[:], out_offset=None,
                in_=is_missing[:].rearrange("(r one) -> r one", one=1),
                in_offset=bass.IndirectOffsetOnAxis(ap=idx_t[:], axis=0))
            nc.gpsimd.indirect_dma_start(
                out=dm_t[:], out_offset=None,
                in_=drug_map[:].rearrange("(r one) -> r one", one=1),
                in_offset=bass.IndirectOffsetOnAxis(ap=tidx_t[:], axis=0))
            pid_t = sb.tile([P, J], i32)
            mi_sc = sb.tile([P, J], i32)
            nc.vector.tensor_scalar_mul(out=pid_t[:], in0=cm_t[:], scalar1=NDRUG)
            nc.vector.tensor_scalar_mul(out=mi_sc[:], in0=mi_t[:], scalar1=100 * NDRUG)
            nc.vector.tensor_tensor(out=pid_t[:], in0=pid_t[:], in1=mi_sc[:], op=ALU.add)
            nc.vector.tensor_tensor(out=pid_t[:], in0=pid_t[:], in1=dm_t[:], op=ALU.add)

            # ======== load weights / params ========
            ident = sb.tile([P, P], f32)
            make_identity(nc, ident[:])
            w1_kt = []
            for kt in range(8):
                t = sbw.tile([P, CEMB], f32, tag=f"w1_{kt}")
                nc.sync.dma_start(out=t[:], in_=W1[kt * P:(kt + 1) * P, :])
                w1_kt.append(t)
            wf1c_kt = []
            for kt in range(8):
                t = sbw.tile([P, HID], f32, tag=f"wf1c_{kt}")
                nc.sync.dma_start(out=t[:], in_=Wf1c[kt * P:(kt + 1) * P, :])
                wf1c_kt.append(t)
            wf1d_sb = sbw.tile([DEMB, HID], f32)
            nc.sync.dma_start(out=wf1d_sb[:], in_=Wf1d[:])
            wf2_k0 = sbw.tile([P, HID], f32)
            wf2_k1 = sbw.tile([HID - P, HID], f32)
            nc.sync.dma_start(out=wf2_k0[:], in_=Wf2[0:P, :])
            nc.sync.dma_start(out=wf2_k1[:], in_=Wf2[P:HID, :])
            wf3_k0 = sbw.tile([P, NDOSES], f32)
            wf3_k1 = sbw.tile([HID - P, NDOSES], f32)
            nc.sync.dma_start(out=wf3_k0[:], in_=Wf3[0:P, :])
            nc.sync.dma_start(out=wf3_k1[:], in_=Wf3[P:HID, :])
            b1_row = sbw.tile([1, CEMB], f32)
            nc.sync.dma_start(out=b1_row[:], in_=b1[:].rearrange("(one n) -> one n", one=1))
            bf1_row = sbw.tile([1, HID], f32)
            nc.sync.dma_start(out=bf1_row[:], in_=bf1[:].rearrange("(one n) -> one n", one=1))
            bf2_c0 = sbw.tile([P, 1], f32)
            bf2_c1 = sbw.tile([HID - P, 1], f32)
            nc.sync.dma_start(out=bf2_c0[:], in_=bf2[0:P].rearrange("(p one) -> p one", one=1))
            nc.sync.dma_start(out=bf2_c1[:], in_=bf2[P:HID].rearrange("(p one) -> p one", one=1))
            bf3_row = sbw.tile([1, NDOSES], f32)
            nc.sync.dma_start(out=bf3_row[:], in_=bf3[:].rearrange("(one n) -> one n", one=1))
            Lsp_sb = sbw.tile([NDOSES - 1, NDOSES], f32)
            nc.sync.dma_start(out=Lsp_sb[:], in_=Lmat[:])
            ones9 = sbw.tile([1, NDOSES], f32)
            nc.vector.memset(ones9[:], 1.0)
            cft_kt = []
            for kt in range(8):
                t = sbw.tile([P, S_SH], f32, tag=f"cft_{kt}")
                nc.sync.dma_start(out=t[:], in_=cfT_s[kt * P:(kt + 1) * P, :])
                cft_kt.append(t)
            me_sb = sb.tile([S_SH, CEMB], f32)
            nc.sync.dma_start(out=me_sb[:], in_=me_s[:])
            flag_c = sb.tile([S_SH, 1], f32)
            nc.sync.dma_start(out=flag_c[:], in_=flag_s[:].rearrange("(p one) -> p one", one=1))
            de_p = []
            for mt in range(2):
                t = sb.tile([P, DEMB], f32, tag=f"de_{mt}")
                nc.sync.dma_start(out=t[:], in_=drug_emb[mt * P:(mt + 1) * P, :])
                de_p.append(t)
            deT_sb = sb.tile([DEMB, NDRUG], f32)
            nc.sync.dma_start(out=deT_sb[:], in_=drug_embT[:])
            ones_row = sbw.tile([1, 512], f32)
            nc.vector.memset(ones_row[:], 1.0)
            ones_s = sbw.tile([1, S_SH], f32)
            nc.vector.memset(ones_s[:], 1.0)

            # ======== P_shard = relu(cf_shard @ W1 + b1)  [25, 1024] ========
            p_sb = sb.tile([S_SH, CEMB], f32)
            for nh in range(2):
                pps = ps_ps.tile([S_SH, 512], f32, tag="pshard")
                for kt in range(8):
                    nc.tensor.matmul(
                        out=pps[:], lhsT=cft_kt[kt][:],
                        rhs=w1_kt[kt][:, nh * 512:(nh + 1) * 512],
                        start=(kt == 0), stop=False)
                nc.tensor.matmul(
                    out=pps[:], lhsT=ones_s[:],
                    rhs=b1_row[:, nh * 512:(nh + 1) * 512],
                    start=False, stop=True)
                nc.scalar.activation(
                    out=p_sb[:, nh * 512:(nh + 1) * 512], in_=pps[:], func=AF.Relu)

            # ======== norms + blend -> Cn [25, 1024] ========
            sq = sb.tile([S_SH, CEMB], f32)
            ssp = sb.tile([S_SH, 1], f32)
            ssm = sb.tile([S_SH, 1], f32)
            nc.scalar.activation(out=sq[:], in_=p_sb[:], func=AF.Square)
            nc.vector.reduce_sum(out=ssp[:], in_=sq[:], axis=mybir.AxisListType.X)
            nc.scalar.activation(out=sq[:], in_=me_sb[:], func=AF.Square)
            nc.vector.reduce_sum(out=ssm[:], in_=sq[:], axis=mybir.AxisListType.X)
            for ss in (ssp, ssm):
                nc.scalar.activation(out=ss[:], in_=ss[:], func=AF.Sqrt)
                nc.vector.tensor_scalar_max(out=ss[:], in0=ss[:], scalar1=EPS)
                nc.vector.reciprocal(out=ss[:], in_=ss[:])
            # r = ssp + (ssm - ssp)*flag ; U = P + (me - P)*flag ; Cn = U*r
            rr = sb.tile([S_SH, 1], f32)
            nc.vector.tensor_tensor(out=rr[:], in0=ssm[:], in1=ssp[:], op=ALU.subtract)
            nc.vector.tensor_tensor(out=rr[:], in0=rr[:], in1=flag_c[:], op=ALU.mult)
            nc.vector.tensor_tensor(out=rr[:], in0=rr[:], in1=ssp[:], op=ALU.add)
            cn_sb = sb.tile([S_SH, CEMB], f32)
            nc.vector.tensor_tensor(out=cn_sb[:], in0=me_sb[:], in1=p_sb[:], op=ALU.subtract)
            nc.vector.tensor_scalar_mul(out=cn_sb[:], in0=cn_sb[:], scalar1=flag_c[:])
            nc.vector.tensor_tensor(out=cn_sb[:], in0=cn_sb[:], in1=p_sb[:], op=ALU.add)
            nc.vector.tensor_scalar_mul(out=cn_sb[:], in0=cn_sb[:], scalar1=rr[:])

            # ======== CnT via PE transpose -> [128, 8*25] ========
            cnt_sb = sb.tile([P, 8 * S_SH], f32)
            for kt in range(8):
                tp = ps_tr.tile([P, S_SH], f32, tag="tr")
                nc.tensor.transpose(
                    out=tp[:], in_=cn_sb[:, kt * P:(kt + 1) * P],
                    identity=ident[:S_SH, :S_SH])
                nc.vector.tensor_copy(
                    out=cnt_sb[:, kt * S_SH:(kt + 1) * S_SH], in_=tp[:])

            # ======== A = Cn @ Wf1c + bf1  [25, 200] ========
            aps = ps_aa.tile([S_SH, HID], f32, tag="a")
            for kt in range(8):
                nc.tensor.matmul(
                    out=aps[:], lhsT=cnt_sb[:, kt * S_SH:(kt + 1) * S_SH],
                    rhs=wf1c_kt[kt][:], start=(kt == 0), stop=False)
            nc.tensor.matmul(out=aps[:], lhsT=ones_s[:], rhs=bf1_row[:],
                             start=False, stop=True)
            a_sb = sb.tile([S_SH, HID], f32)
            nc.vector.tensor_copy(out=a_sb[:], in_=aps[:])

            # AT k-tiles [128, 25], [72, 25]
            at_k0 = sb.tile([P, S_SH], f32)
            at_k1 = sb.tile([HID - P, S_SH], f32)
            for (dst, sl) in ((at_k0, slice(0, P)), (at_k1, slice(P, HID))):
                tp = ps_tr.tile([P, S_SH], f32, tag="tr")
                nc.tensor.transpose(out=tp[:sl.stop - sl.start, :],
                                    in_=a_sb[:, sl], identity=ident[:S_SH, :S_SH])
                nc.vector.tensor_copy(out=dst[:], in_=tp[:sl.stop - sl.start, :])

            # ======== drug side: rd, Bd = Dn @ Wf1d  [256, 200] ========
            bd_mt = []
            for mt in range(2):
                sqd = sb.tile([P, DEMB], f32, tag="sqd")
                rd = sb.tile([P, 1], f32, tag=f"rd_{mt}")
                nc.scalar.activation(out=sqd[:], in_=de_p[mt][:], func=AF.Square)
                nc.vector.reduce_sum(out=rd[:], in_=sqd[:], axis=mybir.AxisListType.X)
                nc.scalar.activation(out=rd[:], in_=rd[:], func=AF.Sqrt)
                nc.vector.tensor_scalar_max(out=rd[:], in0=rd[:], scalar1=EPS)
                nc.vector.reciprocal(out=rd[:], in_=rd[:])
                bps = ps_bd.tile([P, HID], f32, tag="bd")
                nc.tensor.matmul(out=bps[:], lhsT=deT_sb[:, mt * P:(mt + 1) * P],
                                 rhs=wf1d_sb[:], start=True, stop=True)
                bsb = sb.tile([P, HID], f32, tag=f"bd_{mt}")
                nc.vector.tensor_scalar_mul(out=bsb[:], in0=bps[:], scalar1=rd[:])
                bd_mt.append(bsb)

            # BdT k-tiles [128, 256], [72, 256]
            bdt_k0 = sb.tile([P, NDRUG], f32)
            bdt_k1 = sb.tile([HID - P, NDRUG], f32)
            for mt in range(2):
                for (dst, sl) in ((bdt_k0, slice(0, P)), (bdt_k1, slice(P, HID))):
                    tp = ps_tr.tile([P, P], f32, tag="tr")
                    kk = sl.stop - sl.start
                    nc.tensor.transpose(out=tp[:kk, :P], in_=bd_mt[mt][:, sl],
                                        identity=ident[:P, :P])
                    nc.vector.tensor_copy(out=dst[:, mt * P:(mt + 1) * P],
                                          in_=tp[:kk, :P])

            # ======== H1T[k, s*256+dm] = relu(A[s,k] + Bd[dm,k]) ========
            h1t_a = sb.tile([P, NPAIR_SH], f32)
            h1t_b = sb.tile([HID - P, NPAIR_SH], f32)
            for s in range(S_SH):
                for (h1, bdt, at) in ((h1t_a, bdt_k0, at_k0), (h1t_b, bdt_k1, at_k1)):
                    nc.vector.tensor_scalar(
                        out=h1[:, s * NDRUG:(s + 1) * NDRUG], in0=bdt[:],
                        scalar1=at[:, s:s + 1], scalar2=0.0,
                        op0=ALU.add, op1=ALU.max)

        # ======== pair-chunk pipeline ========
        with (
            tc.tile_pool(name="ps_h", bufs=2, space="PSUM") as ps_h,
            tc.tile_pool(name="ps_h1", bufs=1, space="PSUM") as ps_h1,
            tc.tile_pool(name="sbc", bufs=2) as sbc,
        ):
            mu_sb = sb.tile([P, (NPAIR_SH // P) * NDOSES], f32)
            nchunk = (NPAIR_SH + 511) // 512
            for ch in range(nchunk):
                n0 = ch * 512
                nn = min(512, NPAIR_SH - n0)
                sl = slice(0, nn)
                h2t_a = sbc.tile([P, 512], f32, tag="h2ta")
                h2t_b = sbc.tile([HID - P, 512], f32, tag="h2tb")
                gt = sbc.tile([NDOSES - 1, 512], f32, tag="gt")
                base_t = sbc.tile([1, 512], f32, tag="base")
                h2a = ps_h.tile([P, 512], f32, tag="h2a")
                h2b = ps_h.tile([HID - P, 512], f32, tag="h2b")
                nc.tensor.matmul(out=h2a[:, :nn], lhsT=wf2_k0[:, 0:P],
                                 rhs=h1t_a[:, n0:n0 + nn], start=True, stop=False)
                nc.tensor.matmul(out=h2a[:, :nn], lhsT=wf2_k1[:, 0:P],
                                 rhs=h1t_b[:, n0:n0 + nn], start=False, stop=True)
                nc.tensor.matmul(out=h2b[:, :nn], lhsT=wf2_k0[:, P:HID],
                                 rhs=h1t_a[:, n0:n0 + nn], start=True, stop=False)
                nc.tensor.matmul(out=h2b[:, :nn], lhsT=wf2_k1[:, P:HID],
                                 rhs=h1t_b[:, n0:n0 + nn], start=False, stop=True)
                nc.scalar.activation(out=h2t_a[:, sl], in_=h2a[:, :nn],
                                     func=AF.Relu, bias=bf2_c0[:], scale=1.0)
                nc.scalar.activation(out=h2t_b[:, sl], in_=h2b[:, :nn],
                                     func=AF.Relu, bias=bf2_c1[:], scale=1.0)
                # fwd split: softplus-part (doses 1..8) and base (dose 0) as
                # separate partition-0-based tiles
                ftsp = ps_h.tile([NDOSES - 1, 512], f32, tag="ftsp")
                nc.tensor.matmul(out=ftsp[:, :nn], lhsT=wf3_k0[:, 1:NDOSES],
                                 rhs=h2t_a[:, sl], start=True, stop=False)
                nc.tensor.matmul(out=ftsp[:, :nn], lhsT=wf3_k1[:, 1:NDOSES],
                                 rhs=h2t_b[:, sl], start=False, stop=False)
                nc.tensor.matmul(out=ftsp[:, :nn], lhsT=bf3_row[:, 1:NDOSES],
                                 rhs=ones_row[:, :nn], start=False, stop=True)
                ftb = ps_h1.tile([1, 512], f32, tag="ftb")
                nc.tensor.matmul(out=ftb[:, :nn], lhsT=wf3_k0[:, 0:1],
                                 rhs=h2t_a[:, sl], start=True, stop=False)
                nc.tensor.matmul(out=ftb[:, :nn], lhsT=wf3_k1[:, 0:1],
                                 rhs=h2t_b[:, sl], start=False, stop=False)
                nc.tensor.matmul(out=ftb[:, :nn], lhsT=bf3_row[:, 0:1],
                                 rhs=ones_row[:, :nn], start=False, stop=True)
                # softplus = ln(exp(x)+1); base passes through
                nc.scalar.activation(out=gt[:, sl], in_=ftsp[:, :nn], func=AF.Exp)
                nc.scalar.activation(out=gt[:, sl], in_=gt[:, sl],
                                     func=AF.Ln, bias=1.0, scale=1.0)
                nc.vector.tensor_copy(out=base_t[:, sl], in_=ftb[:, :nn])
                # mu natural chunks: mu[p,o] = base[p] + sum_{k<=o-1} sp[k,p]
                nblk = nn // P
                mups = ps_h1.tile([P, 4 * NDOSES], f32, tag="mu")
                for j in range(nblk):
                    bsl = slice(j * P, (j + 1) * P)
                    nc.tensor.matmul(
                        out=mups[:, j * NDOSES:(j + 1) * NDOSES],
                        lhsT=gt[:, bsl], rhs=Lsp_sb[:], start=True, stop=False)
                    nc.tensor.matmul(
                        out=mups[:, j * NDOSES:(j + 1) * NDOSES],
                        lhsT=base_t[:, bsl], rhs=ones9[:], start=False, stop=True)
                nc.vector.tensor_copy(
                    out=mu_sb[:, (ch * 4) * NDOSES:(ch * 4 + nblk) * NDOSES],
                    in_=mups[:, :nblk * NDOSES])

            # ======== stage shard -> AllGather -> final gather ========
            nc.sync.dma_start(
                out=cc_in[:].rearrange("(blk p) o -> p blk o", p=P),
                in_=mu_sb[:].rearrange("p (blk o) -> p blk o", o=NDOSES))
            nc.gpsimd.collective_compute(
                "AllGather", mybir.AluOpType.bypass,
                replica_groups=[list(range(NCORES))],
                ins=[cc_in[:].opt()], outs=[cc_out[:].opt()])
            mu_t = sb.tile([P, J * NDOSES], f32)
            nc.gpsimd.indirect_dma_start(
                out=mu_t[:], out_offset=None, in_=cc_out[:],
                in_offset=bass.IndirectOffsetOnAxis(ap=pid_t[:], axis=0))
            nc.sync.dma_start(
                out=mu_s[:].rearrange("(p j) o -> p (j o)", p=P),
                in_=mu_t[:])

    return _split_sync_waits(nc) if split_waits else nc


def _get_nc():
    if "nc" not in _NC_CACHE:
        _NC_CACHE["nc"] = build_nc()
    return _NC_CACHE["nc"]


def make_in_maps(inputs):
    idx = np.ascontiguousarray(np.asarray(inputs["idx"], np.int32))
    tidx = np.ascontiguousarray(np.asarray(inputs["tidx"], np.int32))
    cell_map = np.ascontiguousarray(np.asarray(inputs["cell_map"], np.int32))
    is_missing = np.ascontiguousarray(np.asarray(inputs["is_missing"], np.int32))
    drug_map = np.ascontiguousarray(np.asarray(inputs["drug_map"], np.int32))
    cf = np.asarray(inputs["cell_features"], np.float32)
    me = np.asarray(inputs["missing_emb"], np.float32)
    de = np.asarray(inputs["drug_emb"], np.float32)
    W1 = np.ascontiguousarray(np.asarray(inputs["W1"], np.float32))
    b1 = np.ascontiguousarray(np.asarray(inputs["b1"], np.float32))
    Wf1 = np.asarray(inputs["Wf1"], np.float32)
    bf1 = np.ascontiguousarray(np.asarray(inputs["bf1"], np.float32))
    Wf2 = np.ascontiguousarray(np.asarray(inputs["Wf2"], np.float32))
    bf2 = np.ascontiguousarray(np.asarray(inputs["bf2"], np.float32))
    Wf3 = np.ascontiguousarray(np.asarray(inputs["Wf3"], np.float32))
    bf3 = np.ascontiguousarray(np.asarray(inputs["bf3"], np.float32))

    zeros_cfT = np.zeros((NFEAT, S_SH), np.float32)
    zeros_me = np.zeros((S_SH, CEMB), np.float32)
    shared = dict(
        cell_map=cell_map, is_missing=is_missing, drug_map=drug_map,
        drug_emb=np.ascontiguousarray(de),
        drug_embT=np.ascontiguousarray(de.T),
        W1=W1, b1=b1,
        Wf1c=np.ascontiguousarray(Wf1[:CEMB, :]),
        Wf1d=np.ascontiguousarray(Wf1[CEMB:, :]),
        bf1=bf1, Wf2=Wf2, bf2=bf2, Wf3=Wf3, bf3=bf3,
    )
    in_maps = []
    for c in range(NCORES):
        m = dict(shared)
        m["idx_s"] = np.ascontiguousarray(idx[c * BS:(c + 1) * BS])
        m["tidx_s"] = np.ascontiguousarray(tidx[c * BS:(c + 1) * BS])
        if c < 4:
            m["cfT_s"] = np.ascontiguousarray(cf[c * S_SH:(c + 1) * S_SH, :].T)
            m["me_s"] = zeros_me
            m["flag_s"] = np.zeros((S_SH,), np.float32)
        else:
            m["cfT_s"] = zeros_cfT
            m["me_s"] = np.ascontiguousarray(me[(c - 4) * S_SH:(c - 3) * S_SH, :])
            m["flag_s"] = np.ones((S_SH,), np.float32)
        in_maps.append(m)
    return in_maps


def kernel(**inputs):
    nc = _get_nc()
    in_maps = make_in_maps(inputs)
    res = run_bass_kernel_spmd(nc, in_maps, core_ids=list(range(NCORES)))
    return np.concatenate([res.results[c]["mu_s"] for c in range(NCORES)], axis=0)
